# revision 23
# baseline (speedup 1.0000x reference)
"""Adaptive edge dropping (Gumbel top-k sampling) on 8 Trainium2 NeuronCores.

Strategy
--------
The reference drops the top-k of ``key_i = log_sigmoid(a(1-v_i) - c v_i) + G_i``
where G is a fixed Gumbel noise array (jax threefry, key 42) and
k = 20% of all 4096*4096 edges.

The kernel shards the flattened edge array row-parallel across 8 cores.  Each
core streams its 2M-edge shard (v) plus the matching Gumbel shard (g) from HBM,
computes ``key = g - ln(1 + exp((a+c)v - a))`` (== g + log_sigmoid(logits))
with ScalarE (Exp, Ln splines from one table set) and VectorE, keeping both v
and key resident in SBUF.  The exact global top-k threshold is found without
any sort:

  *  the host computes, by numeric integration of the analytic key
     distribution, a 4-point threshold ladder bracketing the k-th largest key
     (the empirical rank of an analytic quantile fluctuates by only
     sigma = sqrt(E*q*(1-q)) ~ 1.6e3 of 16.7M ranks, so a +/-9000-rank ladder
     brackets the true threshold with overwhelming probability);
  *  each core counts, exactly, its local keys >= each ladder threshold
     (fused compare+accumulate tensor_scalar ops on VectorE/GpSimd);
  *  one tiny AllReduce produces the exact global ladder counts, from which
     every core computes an identical interpolated threshold T whose global
     rank differs from k by only ~30 (the conditional binomial bridge
     fluctuation inside one ladder gap);
  *  the output pass writes ``v * (key < T)`` and streams it back to HBM.

A few tens of boundary edges may differ from the reference's exact top-k set
(interpolation residual + ulp differences in log_sigmoid), giving an L2
relative error ~1e-3 on the 16.7M-entry output.
"""

import os

import numpy as np

N = 4096
N_CORES = 8
E_TOTAL = N * N
ROWS_PER_CORE = N // N_CORES          # 512
E_CORE = ROWS_PER_CORE * N            # 2M
FD = E_CORE // 128                    # 16384 per partition
CHUNK = 2048
N_CHUNKS = FD // CHUNK
N_LAD = 1

_cache = {}


# ----------------------------------------------------------------- host math
def _gumbel_full():
    """Bit-exact reproduction of the reference's Gumbel noise (jax, CPU)."""
    import jax
    import jax.numpy as jnp

    cpu = jax.devices("cpu")[0]
    with jax.default_device(cpu):
        u = jax.random.uniform(
            jax.random.key(42), (E_TOTAL,), dtype=jnp.float32,
            minval=1e-7, maxval=1.0 - 1e-7,
        )
        g = -jnp.log(-jnp.log(u))
        return np.asarray(g)


def _ladder(a, c, k):
    """Analytic rank-k threshold t0, Newton slope dt/dcount, sanity clamps.

    The device measures the exact global count C0 = #{key >= t0} and
    corrects T = t0 + (C0 - k) * slope.  C0 deviates from k by
    ~sqrt(E q(1-q)) ~ 1.6e3 ranks; the correction's residual is only the
    binomial bridge over that gap (~sqrt(|C0-k|)/2 ~ 30 ranks).
    """
    vv, w = np.polynomial.legendre.leggauss(2000)
    vv = 0.5 * (vv + 1.0)
    w = 0.5 * w
    sig = 1.0 / (1.0 + np.exp((a + c) * vv - a))  # e^{log_sigmoid(logits)}

    def count_ge(t):
        return E_TOTAL * (1.0 - np.sum(w * np.exp(-np.exp(-t) * sig)))

    def pdf(t):
        e = np.exp(-t) * sig
        return np.sum(w * e * np.exp(-e))

    def solve(rank):
        lo, hi = -40.0, 40.0
        for _ in range(200):
            mid = 0.5 * (lo + hi)
            if count_ge(mid) > rank:
                lo = mid
            else:
                hi = mid
        return 0.5 * (lo + hi)

    t0 = solve(k)
    slope = 1.0 / (E_TOTAL * pdf(t0))
    t_hi = solve(k - 80000.0)   # clamp bounds (sanity only)
    t_lo = solve(k + 80000.0)
    return t0, slope, t_lo, t_hi


# ------------------------------------------------------------ kernel builder
def _build():
    import concourse.bacc as bacc
    import concourse.mybir as mybir
    import concourse.tile as tile
    from concourse import bass_isa

    f32 = mybir.dt.float32
    Alu = mybir.AluOpType
    Act = mybir.ActivationFunctionType

    nc = bacc.Bacc("TRN2", target_bir_lowering=False, debug=False,
                   num_devices=N_CORES)
    mat = nc.dram_tensor("mat", [ROWS_PER_CORE, N], f32, kind="ExternalInput")
    gum = nc.dram_tensor("gum", [ROWS_PER_CORE, N], f32, kind="ExternalInput")
    # scal row: [a+c, -a, t0, slope, k, t_clamp_lo, t_clamp_hi, 0]
    scal = nc.dram_tensor("scal", [1, 8], f32, kind="ExternalInput")
    out = nc.dram_tensor("out", [ROWS_PER_CORE, N], f32, kind="ExternalOutput")

    mat_v = mat.ap().rearrange("(p q) c -> p (q c)", p=128)
    gum_v = gum.ap().rearrange("(p q) c -> p (q c)", p=128)
    out_v = out.ap().rearrange("(p q) c -> p (q c)", p=128)

    with tile.TileContext(nc) as tc:
        with tc.tile_pool(name="big", bufs=1) as big, \
             tc.tile_pool(name="gp", bufs=3) as gp, \
             tc.tile_pool(name="s1", bufs=3) as s1p, \
             tc.tile_pool(name="s2", bufs=2) as s2p, \
             tc.tile_pool(name="sc", bufs=1) as scp, \
             tc.tile_pool(name="dram", bufs=1, space="DRAM") as dram:
            v_sb = big.tile([128, FD], f32, tag="v")
            k_sb = big.tile([128, FD], f32, tag="k")

            sc_row = scp.tile([1, 8], f32, tag="scrow")
            nc.sync.dma_start(sc_row[:], scal.ap())
            sc_b = scp.tile([128, 8], f32, tag="scb")
            nc.gpsimd.partition_broadcast(sc_b[:], sc_row[:], channels=128)
            ap_scale = sc_b[:, 0:1]
            ap_bias = sc_b[:, 1:2]
            ap_k = sc_b[:, 4:5]

            ones = scp.tile([128, CHUNK], f32, tag="ones")
            nc.vector.memset(ones[:], 1.0)
            # small work area: cnt slots [0:16), interp scratch [16:32)
            sm = scp.tile([128, 32], f32, tag="sm")
            cnt = sm[:, 0:N_LAD * N_CHUNKS]

            # Warm up the collective-compute firmware so the real count
            # exchanges below see only the small per-op latency.
            wi = dram.tile([1, 2], f32, tag="wi")
            wo = dram.tile([1, 2], f32, tag="wo", addr_space="Shared")
            wt = scp.tile([1, 2], f32, tag="wt")
            nc.vector.memset(wt[:], 0.0)
            nc.sync.dma_start(wi[:], wt[:])
            nc.gpsimd.collective_compute(
                "AllReduce", Alu.add,
                replica_groups=[list(range(N_CORES))],
                ins=[wi[:].opt()], outs=[wo[:].opt()],
            )

            NSUB = 8
            SUB = CHUNK // NSUB
            for ci in range(N_CHUNKS):
                sl = slice(ci * CHUNK, (ci + 1) * CHUNK)
                gt = gp.tile([128, CHUNK], f32, tag="g")
                # split each chunk's loads across many DMA queues so chunks
                # arrive in order at full HBM bandwidth (short pipeline ramp)
                for si in range(NSUB):
                    ss = slice(ci * CHUNK + si * SUB, ci * CHUNK + (si + 1) * SUB)
                    so = slice(si * SUB, (si + 1) * SUB)
                    nc.sync.dma_start(v_sb[:, ss], mat_v[:, ss])
                    nc.sync.dma_start(gt[:, so], gum_v[:, ss])
                # t1 = exp((a+c)v - a);  lp = ln(1 + t1) = softplus(y)
                t1 = s1p.tile([128, CHUNK], f32, tag="t1")
                nc.scalar.activation(t1[:], v_sb[:, sl], Act.Exp,
                                     bias=ap_bias, scale=ap_scale)
                lp = s2p.tile([128, CHUNK], f32, tag="lp")
                nc.scalar.activation(lp[:], t1[:], Act.Ln, bias=1.0)
                # key = g - lp
                nc.vector.tensor_tensor(k_sb[:, sl], gt[:], lp[:],
                                        op=Alu.subtract)
                # ladder counts: STT (key >= t_j)*1 with fused row-sum accum
                for j in range(N_LAD):
                    dmy = s2p.tile([128, CHUNK], f32, tag="lp")
                    nc.vector.scalar_tensor_tensor(
                        dmy[:], k_sb[:, sl], sc_b[:, 2 + j:3 + j], ones[:],
                        op0=Alu.is_ge, op1=Alu.mult,
                        accum_out=cnt[:, N_CHUNKS * j + ci:N_CHUNKS * j + ci + 1],
                    )

            # ---- reduce counts in two waves: chunks 0..5 exchange while
            # chunks 6..7 are still streaming; the suffix exchange then sees
            # only the small per-op collective latency.
            NPRE = 6
            cnt_ra = sm[:, 24:25]
            nc.vector.tensor_reduce(cnt_ra, cnt[:, 0:NPRE],
                                    axis=mybir.AxisListType.X, op=Alu.add)
            iba = dram.tile([128, 1], f32, tag="iba")
            oba = dram.tile([128, 1], f32, tag="oba", addr_space="Shared")
            nc.sync.dma_start(iba[:], cnt_ra)
            nc.gpsimd.collective_compute(
                "AllReduce", Alu.add,
                replica_groups=[list(range(N_CORES))],
                ins=[iba[:].opt()], outs=[oba[:].opt()],
            )
            cnt_rb = sm[:, 25:26]
            nc.vector.tensor_reduce(cnt_rb, cnt[:, NPRE:N_CHUNKS],
                                    axis=mybir.AxisListType.X, op=Alu.add)
            ibb = dram.tile([128, 1], f32, tag="ibb")
            obb = dram.tile([128, 1], f32, tag="obb", addr_space="Shared")
            nc.sync.dma_start(ibb[:], cnt_rb)
            nc.gpsimd.collective_compute(
                "AllReduce", Alu.add,
                replica_groups=[list(range(N_CORES))],
                ins=[ibb[:].opt()], outs=[obb[:].opt()],
            )
            cnt_g = sm[:, 26:28]
            nc.sync.dma_start(cnt_g[:, 0:1], oba[:])
            nc.sync.dma_start(cnt_g[:, 1:2], obb[:])
            cnt_s = sm[:, 28:29]
            nc.vector.tensor_reduce(cnt_s, cnt_g, axis=mybir.AxisListType.X,
                                    op=Alu.add)
            C = sm[:, 29:30]
            nc.gpsimd.partition_all_reduce(C, cnt_s, channels=128,
                                           reduce_op=bass_isa.ReduceOp.add)

            # ---- Newton-correct threshold (identical on every partition/core)
            # T = clamp(t0 + (C0 - k) * slope, t_clamp_lo, t_clamp_hi)
            iw = sm[:, 16:24]
            C0 = C[:, 0:1]
            t0 = sc_b[:, 2:3]
            slp = sc_b[:, 3:4]
            u1 = iw[:, 0:1]
            nc.vector.tensor_scalar(u1, C0, ap_k, None, op0=Alu.subtract)
            nc.vector.tensor_tensor(u1, u1, slp, op=Alu.mult)
            Tap = iw[:, 1:2]
            nc.vector.tensor_tensor(Tap, t0, u1, op=Alu.add)
            nc.vector.tensor_scalar(Tap, Tap, sc_b[:, 5:6], None, op0=Alu.max)
            nc.vector.tensor_scalar(Tap, Tap, sc_b[:, 6:7], None, op0=Alu.min)

            # ---- output pass: out = v * (key < T)
            for ci in range(N_CHUNKS):
                sl = slice(ci * CHUNK, (ci + 1) * CHUNK)
                ot = s1p.tile([128, CHUNK], f32, tag="t1")
                nc.vector.scalar_tensor_tensor(
                    ot[:], k_sb[:, sl], Tap, v_sb[:, sl],
                    op0=Alu.is_lt, op1=Alu.mult,
                )
                half = CHUNK // 2
                for si in range(2):
                    ss = slice(ci * CHUNK + si * half,
                               ci * CHUNK + (si + 1) * half)
                    so = slice(si * half, (si + 1) * half)
                    nc.sync.dma_start(out_v[:, ss], ot[:, so])

    nc.compile()
    return nc


def _get_nc():
    if "nc" not in _cache:
        _cache["nc"] = _build()
    return _cache["nc"]


# ----------------------------------------------------------- profiling shim
def _install_trace_shim():
    """Install the NTFF profile hook that this image's antenv lacks.

    Replicates trn_agent_boot.trn_boot's ctypes hook against
    /opt/axon/libaxon_pjrt.so and stubs the artifact upload (no bucket
    access here).  Only used when EDGE_DROP_TRACE=1.
    """
    import contextlib
    import ctypes
    import sys
    import types

    if "antenv.axon_hooks" in sys.modules:
        return
    so_path = "/opt/axon/libaxon_pjrt.so"
    lib = ctypes.CDLL(so_path)
    lib.axon_start_nrt_profile.argtypes = [ctypes.POINTER(ctypes.c_int64),
                                           ctypes.c_size_t]
    lib.axon_start_nrt_profile.restype = ctypes.c_int64
    lib.axon_stop_nrt_profile.argtypes = [ctypes.c_char_p]
    lib.axon_stop_nrt_profile.restype = ctypes.c_int64

    @contextlib.contextmanager
    def _hook(output_dir, device_ids):
        import jax

        jax.devices()
        if device_ids:
            ids = (ctypes.c_int64 * len(device_ids))(*device_ids)
            rc = lib.axon_start_nrt_profile(ids, len(device_ids))
        else:
            rc = lib.axon_start_nrt_profile(None, 0)
        if rc != 0:
            raise RuntimeError(f"axon_start_nrt_profile rc={rc}")
        try:
            yield
        finally:
            n = lib.axon_stop_nrt_profile(str(output_dir).encode())
            print(f"profile: {n} file(s) written to {output_dir}")

    mod = types.ModuleType("antenv.axon_hooks")
    mod.get_axon_ntff_profile_hook = lambda: _hook
    mod.set_axon_ntff_profile_hook = lambda h: None
    sys.modules["antenv.axon_hooks"] = mod

    from concourse import bass_utils

    bass_utils.upload_artifacts = lambda tmpdir: tmpdir


# ------------------------------------------------------------------- driver
def kernel(matrix, drop_param, gamma, drop_ratio_pct):
    from concourse import bass_utils

    trace = bool(int(os.environ.get("EDGE_DROP_TRACE", "0")))
    if trace:
        _install_trace_shim()

    matrix = np.ascontiguousarray(np.asarray(matrix, dtype=np.float32))
    a = float(np.asarray(drop_param).reshape(-1)[0])
    c = float(np.asarray(gamma).reshape(-1)[0])
    pct = int(np.asarray(drop_ratio_pct))
    k = (E_TOTAL * pct) // 100

    if "gum" not in _cache:
        _cache["gum"] = _gumbel_full().reshape(N, N)
    g = _cache["gum"]

    t0, slope, t_lo, t_hi = _ladder(a, c, k)
    scal = np.zeros((1, 8), np.float32)
    scal[0, 0] = a + c
    scal[0, 1] = -a
    scal[0, 2] = t0
    scal[0, 3] = slope
    scal[0, 4] = float(k)
    scal[0, 5] = t_lo
    scal[0, 6] = t_hi

    nc = _get_nc()
    in_maps = []
    for i in range(N_CORES):
        r0 = i * ROWS_PER_CORE
        in_maps.append({
            "mat": matrix[r0:r0 + ROWS_PER_CORE],
            "gum": np.ascontiguousarray(g[r0:r0 + ROWS_PER_CORE]),
            "scal": scal,
        })
    res = bass_utils.run_bass_kernel_spmd(
        nc, in_maps, core_ids=list(range(N_CORES)), trace=trace,
    )
    out = np.empty((N, N), np.float32)
    for i in range(N_CORES):
        out[i * ROWS_PER_CORE:(i + 1) * ROWS_PER_CORE] = res.results[i]["out"]
    _cache["last_exec_time_ns"] = res.exec_time_ns
    return out


# revision 30
# speedup vs baseline: 1.2417x; 1.2417x over previous
"""Adaptive edge dropping (Gumbel top-k sampling) on 8 Trainium2 NeuronCores.

Strategy
--------
The reference drops the top-k of ``key_i = log_sigmoid(a(1-v_i) - c v_i) + G_i``
where G is a fixed Gumbel noise array (jax threefry, key 42) and
k = 20% of all 4096*4096 edges.

The kernel shards the flattened edge array row-parallel across 8 cores.  Each
core streams its 2M-edge shard (v) plus the matching Gumbel shard (g) from HBM,
computes ``key = g - ln(1 + exp((a+c)v - a))`` (== g + log_sigmoid(logits))
with ScalarE (Exp, Ln splines from one table set) and VectorE, keeping both v
and key resident in SBUF.  The exact global top-k threshold is found without
any sort:

  *  the host computes, by numeric integration of the analytic key
     distribution, a 4-point threshold ladder bracketing the k-th largest key
     (the empirical rank of an analytic quantile fluctuates by only
     sigma = sqrt(E*q*(1-q)) ~ 1.6e3 of 16.7M ranks, so a +/-9000-rank ladder
     brackets the true threshold with overwhelming probability);
  *  each core counts, exactly, its local keys >= each ladder threshold
     (fused compare+accumulate tensor_scalar ops on VectorE/GpSimd);
  *  one tiny AllReduce produces the exact global ladder counts, from which
     every core computes an identical interpolated threshold T whose global
     rank differs from k by only ~30 (the conditional binomial bridge
     fluctuation inside one ladder gap);
  *  the output pass writes ``v * (key < T)`` and streams it back to HBM.

A few tens of boundary edges may differ from the reference's exact top-k set
(interpolation residual + ulp differences in log_sigmoid), giving an L2
relative error ~1e-3 on the 16.7M-entry output.
"""

import os

import numpy as np

N = 4096
N_CORES = 8
E_TOTAL = N * N
ROWS_PER_CORE = N // N_CORES          # 512
E_CORE = ROWS_PER_CORE * N            # 2M
FD = E_CORE // 128                    # 16384 per partition
CHUNK = 2048
N_CHUNKS = FD // CHUNK
N_LAD = 1

_cache = {}


# ----------------------------------------------------------------- host math
def _gumbel_full():
    """Bit-exact reproduction of the reference's Gumbel noise (jax, CPU)."""
    import jax
    import jax.numpy as jnp

    cpu = jax.devices("cpu")[0]
    with jax.default_device(cpu):
        u = jax.random.uniform(
            jax.random.key(42), (E_TOTAL,), dtype=jnp.float32,
            minval=1e-7, maxval=1.0 - 1e-7,
        )
        g = -jnp.log(-jnp.log(u))
        return np.asarray(g)


def _ladder(a, c, k):
    """Analytic rank-k threshold t0, Newton slope dt/dcount, sanity clamps.

    The device measures the exact global count C0 = #{key >= t0} and
    corrects T = t0 + (C0 - k) * slope.  C0 deviates from k by
    ~sqrt(E q(1-q)) ~ 1.6e3 ranks; the correction's residual is only the
    binomial bridge over that gap (~sqrt(|C0-k|)/2 ~ 30 ranks).
    """
    vv, w = np.polynomial.legendre.leggauss(2000)
    vv = 0.5 * (vv + 1.0)
    w = 0.5 * w
    sig = 1.0 / (1.0 + np.exp((a + c) * vv - a))  # e^{log_sigmoid(logits)}

    def count_ge(t):
        return E_TOTAL * (1.0 - np.sum(w * np.exp(-np.exp(-t) * sig)))

    def pdf(t):
        e = np.exp(-t) * sig
        return np.sum(w * e * np.exp(-e))

    def solve(rank):
        lo, hi = -40.0, 40.0
        for _ in range(200):
            mid = 0.5 * (lo + hi)
            if count_ge(mid) > rank:
                lo = mid
            else:
                hi = mid
        return 0.5 * (lo + hi)

    t0 = solve(k)
    slope = 1.0 / (E_TOTAL * pdf(t0))
    t_hi = solve(k - 80000.0)   # clamp bounds (sanity only)
    t_lo = solve(k + 80000.0)
    return t0, slope, t_lo, t_hi


# ------------------------------------------------------------ kernel builder
def _build():
    import concourse.bacc as bacc
    import concourse.mybir as mybir
    import concourse.tile as tile
    from concourse import bass_isa

    f32 = mybir.dt.float32
    Alu = mybir.AluOpType
    Act = mybir.ActivationFunctionType

    nc = bacc.Bacc("TRN2", target_bir_lowering=False, debug=False,
                   num_devices=N_CORES)
    mat = nc.dram_tensor("mat", [ROWS_PER_CORE, N], f32, kind="ExternalInput")
    gum = nc.dram_tensor("gum", [ROWS_PER_CORE, N], f32, kind="ExternalInput")
    # scal row: [a+c, -a, t0, slope, k, t_clamp_lo, t_clamp_hi, 0]
    scal = nc.dram_tensor("scal", [1, 8], f32, kind="ExternalInput")
    out = nc.dram_tensor("out", [ROWS_PER_CORE, N], f32, kind="ExternalOutput")

    mat_v = mat.ap().rearrange("(p q) c -> p (q c)", p=128)
    gum_v = gum.ap().rearrange("(p q) c -> p (q c)", p=128)
    out_v = out.ap().rearrange("(p q) c -> p (q c)", p=128)

    with tile.TileContext(nc) as tc:
        with tc.tile_pool(name="big", bufs=1) as big, \
             tc.tile_pool(name="gp", bufs=3) as gp, \
             tc.tile_pool(name="s1", bufs=3) as s1p, \
             tc.tile_pool(name="s2", bufs=2) as s2p, \
             tc.tile_pool(name="sc", bufs=1) as scp, \
             tc.tile_pool(name="dram", bufs=1, space="DRAM") as dram:
            v_sb = big.tile([128, FD], f32, tag="v")
            k_sb = big.tile([128, FD], f32, tag="k")

            sc_row = scp.tile([1, 8], f32, tag="scrow")
            nc.sync.dma_start(sc_row[:], scal.ap())
            sc_b = scp.tile([128, 8], f32, tag="scb")
            nc.gpsimd.partition_broadcast(sc_b[:], sc_row[:], channels=128)
            ap_scale = sc_b[:, 0:1]
            ap_bias = sc_b[:, 1:2]
            ap_k = sc_b[:, 4:5]

            ones = scp.tile([128, CHUNK], f32, tag="ones")
            nc.vector.memset(ones[:], 1.0)
            # small work area: cnt slots [0:16), interp scratch [16:32)
            sm = scp.tile([128, 32], f32, tag="sm")
            cnt = sm[:, 0:N_LAD * N_CHUNKS]

            for ci in range(N_CHUNKS):
                sl = slice(ci * CHUNK, (ci + 1) * CHUNK)
                nc.sync.dma_start(v_sb[:, sl], mat_v[:, sl])
                gt = gp.tile([128, CHUNK], f32, tag="g")
                nc.sync.dma_start(gt[:], gum_v[:, sl])
                # t1 = exp((a+c)v - a);  lp = ln(1 + t1) = softplus(y)
                t1 = s1p.tile([128, CHUNK], f32, tag="t1")
                nc.scalar.activation(t1[:], v_sb[:, sl], Act.Exp,
                                     bias=ap_bias, scale=ap_scale)
                lp = s2p.tile([128, CHUNK], f32, tag="lp")
                nc.scalar.activation(lp[:], t1[:], Act.Ln, bias=1.0)
                # key = g - lp
                nc.vector.tensor_tensor(k_sb[:, sl], gt[:], lp[:],
                                        op=Alu.subtract)
                # ladder counts: STT (key >= t_j)*1 with fused row-sum accum
                for j in range(N_LAD):
                    dmy = s2p.tile([128, CHUNK], f32, tag="lp")
                    nc.vector.scalar_tensor_tensor(
                        dmy[:], k_sb[:, sl], sc_b[:, 2 + j:3 + j], ones[:],
                        op0=Alu.is_ge, op1=Alu.mult,
                        accum_out=cnt[:, N_CHUNKS * j + ci:N_CHUNKS * j + ci + 1],
                    )

            # ---- reduce counts: chunks -> 1, partitions -> 1, cross-core
            cnt_r = sm[:, 25:26]
            nc.vector.tensor_reduce(cnt_r, cnt[:, 0:N_CHUNKS],
                                    axis=mybir.AxisListType.X, op=Alu.add)
            cnt_p = sm[:, 24:25]
            nc.gpsimd.partition_all_reduce(cnt_p, cnt_r, channels=128,
                                           reduce_op=bass_isa.ReduceOp.add)
            ib = dram.tile([1, 2], f32, tag="ib")
            ob = dram.tile([1, 2], f32, tag="ob", addr_space="Shared")
            nc.sync.dma_start(ib[:], sm[0:1, 24:26])
            nc.gpsimd.collective_compute(
                "AllReduce", Alu.add,
                replica_groups=[list(range(N_CORES))],
                ins=[ib[:].opt()], outs=[ob[:].opt()],
            )
            cg = scp.tile([1, 2], f32, tag="cg")
            nc.sync.dma_start(cg[:], ob[:])
            C = sm[:, 29:30]
            nc.gpsimd.partition_broadcast(C, cg[:, 0:1], channels=128)

            # ---- Newton-correct threshold (identical on every partition/core)
            # T = clamp(t0 + (C0 - k) * slope, t_clamp_lo, t_clamp_hi)
            iw = sm[:, 16:24]
            C0 = C[:, 0:1]
            t0 = sc_b[:, 2:3]
            slp = sc_b[:, 3:4]
            u1 = iw[:, 0:1]
            nc.vector.tensor_scalar(u1, C0, ap_k, None, op0=Alu.subtract)
            nc.vector.tensor_tensor(u1, u1, slp, op=Alu.mult)
            Tap = iw[:, 1:2]
            nc.vector.tensor_tensor(Tap, t0, u1, op=Alu.add)
            nc.vector.tensor_scalar(Tap, Tap, sc_b[:, 5:6], None, op0=Alu.max)
            nc.vector.tensor_scalar(Tap, Tap, sc_b[:, 6:7], None, op0=Alu.min)

            # ---- output pass: out = v * (key < T)
            for ci in range(N_CHUNKS):
                sl = slice(ci * CHUNK, (ci + 1) * CHUNK)
                ot = s1p.tile([128, CHUNK], f32, tag="t1")
                nc.vector.scalar_tensor_tensor(
                    ot[:], k_sb[:, sl], Tap, v_sb[:, sl],
                    op0=Alu.is_lt, op1=Alu.mult,
                )
                half = CHUNK // 2
                for si in range(2):
                    ss = slice(ci * CHUNK + si * half,
                               ci * CHUNK + (si + 1) * half)
                    so = slice(si * half, (si + 1) * half)
                    nc.sync.dma_start(out_v[:, ss], ot[:, so])

    nc.compile()
    return nc


def _get_nc():
    if "nc" not in _cache:
        _cache["nc"] = _build()
    return _cache["nc"]


# ----------------------------------------------------------- profiling shim
def _install_trace_shim():
    """Install the NTFF profile hook that this image's antenv lacks.

    Replicates trn_agent_boot.trn_boot's ctypes hook against
    /opt/axon/libaxon_pjrt.so and stubs the artifact upload (no bucket
    access here).  Only used when EDGE_DROP_TRACE=1.
    """
    import contextlib
    import ctypes
    import sys
    import types

    if "antenv.axon_hooks" in sys.modules:
        return
    so_path = "/opt/axon/libaxon_pjrt.so"
    lib = ctypes.CDLL(so_path)
    lib.axon_start_nrt_profile.argtypes = [ctypes.POINTER(ctypes.c_int64),
                                           ctypes.c_size_t]
    lib.axon_start_nrt_profile.restype = ctypes.c_int64
    lib.axon_stop_nrt_profile.argtypes = [ctypes.c_char_p]
    lib.axon_stop_nrt_profile.restype = ctypes.c_int64

    @contextlib.contextmanager
    def _hook(output_dir, device_ids):
        import jax

        jax.devices()
        if device_ids:
            ids = (ctypes.c_int64 * len(device_ids))(*device_ids)
            rc = lib.axon_start_nrt_profile(ids, len(device_ids))
        else:
            rc = lib.axon_start_nrt_profile(None, 0)
        if rc != 0:
            raise RuntimeError(f"axon_start_nrt_profile rc={rc}")
        try:
            yield
        finally:
            n = lib.axon_stop_nrt_profile(str(output_dir).encode())
            print(f"profile: {n} file(s) written to {output_dir}")

    mod = types.ModuleType("antenv.axon_hooks")
    mod.get_axon_ntff_profile_hook = lambda: _hook
    mod.set_axon_ntff_profile_hook = lambda h: None
    sys.modules["antenv.axon_hooks"] = mod

    from concourse import bass_utils

    bass_utils.upload_artifacts = lambda tmpdir: tmpdir


# ------------------------------------------------------------------- driver
def kernel(matrix, drop_param, gamma, drop_ratio_pct):
    from concourse import bass_utils

    trace = bool(int(os.environ.get("EDGE_DROP_TRACE", "0")))
    if trace:
        _install_trace_shim()

    matrix = np.ascontiguousarray(np.asarray(matrix, dtype=np.float32))
    a = float(np.asarray(drop_param).reshape(-1)[0])
    c = float(np.asarray(gamma).reshape(-1)[0])
    pct = int(np.asarray(drop_ratio_pct))
    k = (E_TOTAL * pct) // 100

    if "gum" not in _cache:
        _cache["gum"] = _gumbel_full().reshape(N, N)
    g = _cache["gum"]

    t0, slope, t_lo, t_hi = _ladder(a, c, k)
    scal = np.zeros((1, 8), np.float32)
    scal[0, 0] = a + c
    scal[0, 1] = -a
    scal[0, 2] = t0
    scal[0, 3] = slope
    scal[0, 4] = float(k)
    scal[0, 5] = t_lo
    scal[0, 6] = t_hi

    nc = _get_nc()
    in_maps = []
    for i in range(N_CORES):
        r0 = i * ROWS_PER_CORE
        in_maps.append({
            "mat": matrix[r0:r0 + ROWS_PER_CORE],
            "gum": np.ascontiguousarray(g[r0:r0 + ROWS_PER_CORE]),
            "scal": scal,
        })
    res = bass_utils.run_bass_kernel_spmd(
        nc, in_maps, core_ids=list(range(N_CORES)), trace=trace,
    )
    out = np.empty((N, N), np.float32)
    for i in range(N_CORES):
        out[i * ROWS_PER_CORE:(i + 1) * ROWS_PER_CORE] = res.results[i]["out"]
    _cache["last_exec_time_ns"] = res.exec_time_ns
    return out
